# revision 26
# baseline (speedup 1.0000x reference)
"""Spiking transformer block (SpikingRetention + spiking MLP) on 8 Trainium2
cores. Data-parallel over B=8 (one batch element per NeuronCore).

Key design (v2):
- Binary spikes are exact in fp8e4, enabling DoubleRow (double-pumped) PE
  matmuls at 0.5 cycles/row:
  * scores q.T k: stride-0 dim-2 APs compute 2*(k.T q) exactly; the decay
    matrix folds the 0.5.
  * retention out: real 2-chunk DoubleRow over m-tiles into per-head-pair
    [64, 2, N] psum (DR dst must start at partition 0 on HW).
  * proj: retention spikes are [64(d), 2(head), N] tiles; each DR contracts a
    128-channel head pair; hi and residual weight streams are separate DRs.
  * fc2: weights packed as [Q(W s), e4m3-residual] chunk pairs with a
    stride-0 spike ifmap. Per-channel scales s_c keep quantization ~2^-4.
- LIF carry-adds ride the PE as scaled-identity accumulation matmuls into the
  next wave's psum group.
- LIF per step: Act extract (A = 2^(t-1) psum + b~); spike on DVE (is_ge) or
  Act (Relu+Sign); stage-A carry = A * (A < thr) computed as complement mask
  (DVE is_lt -> fp16) times A on Pool (gpsimd cannot touch PSUM or run
  tensor_scalar); stage-B carry = DVE scalar_tensor_tensor.

Membrane algebra: A_t = 2^t u_t = A^r_{t-1} + 2^(t-1)(Wx_t + b). Carry
C = A (A < th 2^t); the consuming wave-t psum gets 2^-(t-1) I @ C. proj/fc2
run entirely in per-channel-scaled units (psum, bias, threshold, carry all
scaled by s_c), so no rescale is ever needed.
"""

from contextlib import ExitStack

import numpy as np
import ml_dtypes

import concourse.bacc as bacc
import concourse.tile as tile
from concourse import mybir
from concourse.bass_utils import run_bass_kernel_spmd

f32 = mybir.dt.float32
f32r = mybir.dt.float32r
fp16 = mybir.dt.float16
fp8 = mybir.dt.float8e4
Alu = mybir.AluOpType
Act = mybir.ActivationFunctionType
DR = mybir.MatmulPerfMode.DoubleRow

E4 = ml_dtypes.float8_e4m3

T, B, N, C = 4, 8, 512, 512
HID = 2048
H = 8
EPS = 1e-5
NT = N // 128
CT = C // 128
HT = HID // 128

_CACHE = {}


def _dr2(ap):
    """[p, f] -> [p, 2(stride0), f] for stride-0 DoubleRow operands."""
    p, fr = ap.shape
    return ap.unsqueeze(1).broadcast_to([p, 2, fr])


def _build():
    nc = bacc.Bacc("TRN2", target_bir_lowering=False, debug=False)

    xb = nc.declare_dram_parameter("xb", [T, CT, 128, N], f32r, isOutput=False)
    wqkv_e = nc.declare_dram_parameter("wqkv", [128, 3 * CT, C], f32r,
                                       isOutput=False)
    pw_e = nc.declare_dram_parameter("pw", [64, CT, 2, 2, C], fp8,
                                     isOutput=False)
    w1_e = nc.declare_dram_parameter("w1", [128, CT, HID], fp16, isOutput=False)
    w2_e = nc.declare_dram_parameter("w2", [128, HT, 2, C], fp8, isOutput=False)
    bias_e = nc.declare_dram_parameter("biases", [128, T, 32], f32,
                                       isOutput=False)
    thr_e = nc.declare_dram_parameter("thrs", [128, T, 8], f32, isOutput=False)
    vb_e = nc.declare_dram_parameter("vrow", [1, C + 128], f32r, isOutput=False)
    dmat_e = nc.declare_dram_parameter("dmat", [H, 128, NT, N], fp16,
                                       isOutput=False)
    idm_e = nc.declare_dram_parameter("idm", [128, 3, 128], f32r,
                                      isOutput=False)
    out_e = nc.declare_dram_parameter("out", [T, CT, 128, N], fp16,
                                      isOutput=True)

    DVE = nc.vector
    POOL = nc.gpsimd
    ACT = nc.scalar

    with tile.TileContext(nc) as tc, ExitStack() as ctx:
        pers = ctx.enter_context(tc.tile_pool(name="pers", bufs=1))
        work = ctx.enter_context(tc.tile_pool(name="work", bufs=1))
        xa_pool = ctx.enter_context(tc.tile_pool(name="xa_pool", bufs=1))
        spk_o_pool = ctx.enter_context(tc.tile_pool(name="spk_o_pool", bufs=1))
        wmlp_pool = ctx.enter_context(tc.tile_pool(name="wmlp_pool", bufs=1))
        pwt = wmlp_pool.tile([64, CT, 2, 2, C], fp8, name="w_pw")

        ball = pers.tile([128, T, 32], f32, name="ball")
        thrt = pers.tile([128, T, 8], f32, name="thrt")
        vrow = pers.tile([1, C + 128], f32r, name="vrow")
        idmt = pers.tile([128, 3, 128], f32r, name="idmt")
        ACT.dma_start(ball[:], bias_e[:, :, :])
        ACT.dma_start(vrow[:], vb_e[:, :])
        bias_sb = {"qb": ball[:, :, 0:4], "kb": ball[:, :, 4:8],
                   "pb": ball[:, :, 8:12], "b2": ball[:, :, 12:16],
                   "b1": ball[:, :, 16:32]}
        thr_sb = {"pb": thrt[:, :, 0:4], "b2": thrt[:, :, 4:8]}
        nthr = {}
        for tv in (1.0, 2.0, 4.0, 8.0, 16.0):
            tt_ = pers.tile([128, 1], f32, name=f"nthr{int(tv)}")
            nc.vector.memset(tt_[:], -tv)
            nthr[tv] = tt_
        vbrow = vrow[:, 0:C]
        ones128 = vrow[:, C:C + 128]
        ids = {tt: idmt[:, tt - 2, :] for tt in (2, 3, 4)}

        os_ = {}
        decay_rr = [0]

        # ---------------- LIF helpers ----------------
        def act_spike(st_ap, src, tv, shape=None, p0=128, pool=None):
            # spike via Act (Relu with negated threshold, then Sign)
            shape = shape or [128, 512]
            rl = pool.tile(shape, f32, name="lifrl",
                           tag="lifrl" if p0 == 128 else "lifrlr", bufs=2)
            ACT.activation(rl[:], src, Act.Relu, bias=nthr[tv][0:p0, 0:1])
            ACT.activation(st_ap, rl[:], Act.Sign)

        def emit_spikes(specs):
            for (src, thr, st, cp, ctag, cdst, ckey) in specs:
                if st is not None:
                    DVE.tensor_scalar(st[:], src, thr, None, Alu.is_ge)

        def carry_sbar(src, thr, cp, ctag, cdst, ckey, shape=None):
            # stage-A carry: sbar = (A < thr) fp16 on DVE, C = A*sbar on Pool
            shape = shape or [128, 512]
            p0 = shape[0]
            sb = cp.tile(shape, fp16, name="sbar",
                         tag="sbar" if p0 == 128 else "sbarr", bufs=3)
            DVE.tensor_scalar(sb[:], src, thr, None, Alu.is_lt)
            cn = cp.tile(shape, f32r, name="lifC", tag=ctag, bufs=1)
            POOL.tensor_tensor(cn[:], src, sb[:], Alu.mult)
            cdst[ckey] = cn

        def emit_carries_stt(specs):
            # stage-B carry on DVE: C = (A < thr) * A in one op
            for (src, thr, st, cp, ctag, cdst, ckey) in specs:
                if ctag is not None:
                    cn = cp.tile([128, 512], f32r, name="lifC", tag=ctag,
                                 bufs=1)
                    DVE.scalar_tensor_tensor(cn[:], src, thr, src,
                                             Alu.is_lt, Alu.mult)
                    cdst[ckey] = cn

        # =========== stage A: qkv + retention, t-outer wavefront ===========
        with tc.tile_pool(name="wqkv_pool", bufs=1) as wqkv_pool, \
             tc.tile_pool(name="spk_pool", bufs=1) as spk_pool, \
             tc.tile_pool(name="carry_pool", bufs=1) as carry_pool, \
             tc.tile_pool(name="dm_pool", bufs=1) as dm_pool, \
             tc.tile_pool(name="spool", bufs=1) as spool, \
             tc.tile_pool(name="psA", bufs=1, space="PSUM") as psA:
            wqkv_t = wqkv_pool.tile([128, 3 * CT, C], f32r, name="w_qkv")
            # startup: interleave x wave-1 chunks with qw chunks so the first
            # matmul can start after ~0.5MB of DMA; all on the Pool SWDGE
            # queue (served in emission order by the DMA device).
            xwt = xa_pool.tile([128, CT, N], f32r, name="xT", tag="xT", bufs=2)
            for kt in range(CT):
                nc.sync.dma_start(xwt[:, kt, :], xb[0, kt])
                nc.sync.dma_start(wqkv_t[:, kt, :], wqkv_e[:, kt, :])
            for kt in range(CT, 3 * CT):
                nc.sync.dma_start(wqkv_t[:, kt, :], wqkv_e[:, kt, :])
            wq = {nm: wqkv_t[:, i * CT:(i + 1) * CT, :]
                  for i, nm in enumerate(("qw", "kw", "vw"))}
            dmt = dm_pool.tile([128, H, NT, N], fp16, name="dmt")
            dms = [dmt[:, h] for h in range(H)]

            cq = {}     # carries for q/k/v chains, keyed (nm, ot)
            c_ret = {}  # retention carries per hp

            def ret_scores(hp, qs_p, ks_p, sdst):
                # per head pair: 8 stride-0 DR matmuls + 4 decay multiplies
                h0, h1 = 2 * hp, 2 * hp + 1
                for half in range(2):
                    ps0 = psA.tile([128, 2, N], f32, name="sc0", tag="sc0",
                                   bufs=1)
                    ps1 = psA.tile([128, 2, N], f32, name="sc1", tag="sc1",
                                   bufs=1)
                    for j in range(2):
                        mt = 2 * half + j
                        nc.tensor.matmul(
                            ps0[:, j, :],
                            _dr2(ks_p[hp][0:64, mt * 128:(mt + 1) * 128]),
                            _dr2(qs_p[hp][0:64, :]),
                            start=True, stop=True, perf_mode=DR)
                        nc.tensor.matmul(
                            ps1[:, j, :],
                            _dr2(ks_p[hp][64:128, mt * 128:(mt + 1) * 128]),
                            _dr2(qs_p[hp][64:128, :]),
                            start=True, stop=True, perf_mode=DR)
                    def decay(ps, h):
                        s_ = spool.tile([128, 2, N], fp8, name="sd",
                                        tag=f"sd{decay_rr[0] % 3}", bufs=2)
                        if decay_rr[0] % 6 == 5:
                            # offload via Act copy (scores are small ints,
                            # exact in fp16) + Pool multiply
                            cpy = spool.tile([128, 2, N], fp16, name="scp",
                                             tag="scp", bufs=2)
                            ACT.activation(cpy[:], ps[:], Act.Copy, bias=0.0,
                                           scale=1.0)
                            POOL.tensor_tensor(
                                s_[:], cpy[:],
                                dms[h][:, 2 * half:2 * half + 2, :], Alu.mult)
                        else:
                            DVE.tensor_tensor(
                                s_[:], ps[:],
                                dms[h][:, 2 * half:2 * half + 2, :], Alu.mult)
                        decay_rr[0] += 1
                        return s_
                    s0 = decay(ps0, h0)
                    s1 = decay(ps1, h1)
                    sdst[hp, half] = (s0, s1)

            def ret_out(hp, sdst, vt_p, t_r):
                h0, h1 = 2 * hp, 2 * hp + 1
                pso = psA.tile([64, 2, N], f32, name="pso", tag="pso", bufs=1)
                has_c = (hp in c_ret)
                for half in range(2):
                    s0, s1 = sdst.pop((hp, half))
                    last = (half == 1) and not has_c
                    nc.tensor.matmul(
                        pso[:, 0, :],
                        vt_p[:, 2 * half:2 * half + 2,
                             h0 * 64:(h0 + 1) * 64],
                        s0[:], start=(half == 0), stop=last, perf_mode=DR)
                    nc.tensor.matmul(
                        pso[:, 1, :],
                        vt_p[:, 2 * half:2 * half + 2,
                             h1 * 64:(h1 + 1) * 64],
                        s1[:], start=(half == 0), stop=last, perf_mode=DR)
                if has_c:
                    cr = c_ret[hp]
                    for j in range(2):
                        nc.tensor.matmul(pso[:, j, :], ids[t_r][0:64, 0:64],
                                         cr[:, j, :], start=False, stop=True)
                st = spk_o_pool.tile([64, 2, N], fp8, name="spk_os",
                                     tag="spk_os", bufs=16)
                os_[t_r - 1, hp] = st
                A = spool.tile([64, 2, 512], f32, name="lifAr", tag="lifAr",
                               bufs=2)
                ACT.activation(A[:], pso[:], Act.Copy, bias=0.0,
                               scale=float(2.0 ** (t_r - 1)))
                act_spike(st[:], A[:], float(2.0 ** (t_r - 1)),
                          shape=[64, 2, 512], p0=64, pool=spool)
                if t_r < T:
                    carry_sbar(A[:], float(2.0 ** (t_r - 1)), spool,
                               f"c_o{hp}", c_ret, hp, shape=[64, 2, 512])

            prev = None
            xw_next = None
            for t in range(1, T + 1):
                if t > 1:
                    xwt = xw_next
                xw = {ct: xwt[:, ct, :] for ct in range(CT)}
                qs_c = {}
                ks_c = {}
                vt = spk_pool.tile([128, NT, C], fp8, name="vn", tag="vn",
                                   bufs=2)

                def emit_qk(nm, bnm, dst, ot, t=t):
                    ps = psA.tile([128, N], f32, name="psq", tag="psq", bufs=2)
                    cin = cq.get((nm, ot))
                    for kt in range(CT):
                        nc.tensor.matmul(
                            ps[:], wq[nm][:, kt, ot * 128:(ot + 1) * 128],
                            xw[kt], start=(kt == 0),
                            stop=(kt == CT - 1) and cin is None)
                    if cin is not None:
                        nc.tensor.matmul(ps[:], ids[t], cin[:],
                                         start=False, stop=True)
                    A = work.tile([128, 512], f32, name="lifA", tag="lifA",
                                  bufs=6)
                    ACT.activation(A[:], ps[:], Act.Identity,
                                   bias=bias_sb[bnm][:, t - 1, ot:ot + 1],
                                   scale=float(2.0 ** (t - 1)))
                    st = spk_pool.tile([128, N], fp8, name=f"spk_{nm}",
                                       tag=f"spk_{nm}", bufs=8)
                    dst[ot] = st
                    DVE.tensor_scalar(st[:], A[:], float(2.0 ** t), None,
                                      Alu.is_ge)
                    if t < T:
                        carry_sbar(A[:], float(2.0 ** t), carry_pool,
                                   f"c_{nm}{ot}", cq, (nm, ot))

                def emit_v(nt, t=t):
                    ps = psA.tile([128, C], f32, name="psv", tag="psq", bufs=2)
                    cin = cq.get(("vw", nt))
                    for kt in range(CT):
                        nc.tensor.matmul(ps[:],
                                         xw[kt][:, nt * 128:(nt + 1) * 128],
                                         wq["vw"][:, kt, :],
                                         start=(kt == 0), stop=False)
                    nc.tensor.matmul(ps[:], ones128, vbrow,
                                     start=False, stop=cin is None)
                    if cin is not None:
                        nc.tensor.matmul(ps[:], ids[t], cin[:],
                                         start=False, stop=True)
                    A = work.tile([128, 512], f32, name="lifA", tag="lifA",
                                  bufs=6)
                    ACT.activation(A[:], ps[:], Act.Copy, bias=0.0,
                                   scale=float(2.0 ** (t - 1)))
                    DVE.tensor_scalar(vt[:, nt, :], A[:], float(2.0 ** t),
                                      None, Alu.is_ge)
                    if t < T:
                        carry_sbar(A[:], float(2.0 ** t), carry_pool,
                                   f"c_vw{nt}", cq, ("vw", nt))

                groups = [lambda ot=ot: emit_qk("qw", "qb", qs_c, ot)
                          for ot in range(CT)]
                groups += [lambda ot=ot: emit_qk("kw", "kb", ks_c, ot)
                           for ot in range(CT)]
                groups += [lambda nt=nt: emit_v(nt) for nt in range(NT)]

                if prev is not None:
                    qs_p, ks_p, vt_p = prev
                    sd = {}
                    order = [0, 1, 2, ("s", 0), 3, 4, ("o", 0), ("s", 1),
                             5, 6, ("o", 1), ("s", 2), 7, 8, ("o", 2),
                             ("s", 3), 9, 10, ("o", 3), 11]
                    for item in order:
                        if isinstance(item, int):
                            groups[item]()
                        elif item[0] == "s":
                            ret_scores(item[1], qs_p, ks_p, sd)
                        else:
                            ret_out(item[1], sd, vt_p, t - 1)
                else:
                    for g in groups:
                        g()
                if t < T:  # prefetch next wave's x (SP hwdge queue: free)
                    xw_next = xa_pool.tile([128, CT, N], f32r, name="xT",
                                           tag="xT", bufs=2)
                    for kt in range(CT):
                        nc.sync.dma_start(xw_next[:, kt, :], xb[t, kt])
                if t == 1:  # decay matrices after wave-2 x
                    for hp in range(4):
                        nc.sync.dma_start(
                            dmt[:, 2 * hp:2 * hp + 2],
                            dmat_e.rearrange("h p nt n -> p h nt n")
                            [:, 2 * hp:2 * hp + 2])
                if t == 1:  # proj weights are small; land them early
                    ACT.dma_start(idmt[:], idm_e[:, :, :])
                    ACT.dma_start(thrt[:], thr_e[:, :, :])
                    nc.sync.dma_start(pwt[:], pw_e[:, :, :, :, :])
                prev = (qs_c, ks_c, vt)
            # final retention wave (t = T)
            qs_p, ks_p, vt_p = prev
            sd = {}
            for hp in range(H // 2):
                ret_scores(hp, qs_p, ks_p, sd)
                ret_out(hp, sd, vt_p, T)

        # =========== stage B: proj + MLP + output ===========
        with tc.tile_pool(name="wmlp2", bufs=1) as wmlp2, \
             tc.tile_pool(name="mwork", bufs=1) as mwork, \
             tc.tile_pool(name="xtin_pool", bufs=1) as xtin_pool, \
             tc.tile_pool(name="psM", bufs=1, space="PSUM") as psM:
            w1t = wmlp2.tile([128, CT, HID], fp16, name="w_w1")
            w2t = wmlp2.tile([128, HT, 2, C], fp8, name="w_w2")
            xin1 = xtin_pool.tile([128, CT, N], f32r, name="xtin",
                                  tag="xtin", bufs=2)
            for kt in range(CT):
                nc.sync.dma_start(xin1[:, kt, :], xb[0, kt])
            for kt in range(CT):
                nc.sync.dma_start(w1t[:, kt], w1_e[:, kt])
            for ktp in range(4):
                nc.sync.dma_start(w2t[:, 4 * ktp:4 * ktp + 4],
                                  w2_e[:, 4 * ktp:4 * ktp + 4])
            cp = {}
            c1 = {}
            c2 = {}
            x2_all = {}
            ht_all = {}

            def fc2_wave(t):
                htile = ht_all.pop(t)
                x2 = x2_all.pop(t)
                last = (t == T)
                sts = []
                for ot in range(CT):
                    ps = psM.tile([128, N], f32, name="psf2", tag="psf2",
                                  bufs=2)
                    cin = c2.get(ot)
                    for kt in range(HT):
                        nc.tensor.matmul(
                            ps[:], w2t[:, kt, :, ot * 128:(ot + 1) * 128],
                            _dr2(htile[kt][:]), start=(kt == 0),
                            stop=(kt == HT - 1) and cin is None, perf_mode=DR)
                    if cin is not None:
                        nc.tensor.matmul(ps[:], ids[t], cin[:],
                                         start=False, stop=True)
                    A = work.tile([128, 512], f32, name="lifA", tag="lifA",
                                  bufs=6)
                    ACT.activation(A[:], ps[:], Act.Identity,
                                   bias=bias_sb["b2"][:, t - 1, ot:ot + 1],
                                   scale=float(2.0 ** (t - 1)))
                    st = mwork.tile([128, N], fp16, name="spk_m", tag="spk_m",
                                    bufs=2)
                    sts.append(st)
                    spec = (A[:], thr_sb["b2"][:, t - 1, ot:ot + 1], st,
                            mwork, f"c2_{ot}" if t < T else None, c2, ot)
                    emit_spikes([spec])
                    emit_carries_stt([spec])
                outb = mwork.tile([128, CT, N], fp16, name="outb", tag="outb",
                                  bufs=1)
                for ot in range(CT):
                    DVE.tensor_tensor(outb[:, ot, :], x2[ot], sts[ot][:],
                                      Alu.add)
                    if last:
                        nc.sync.dma_start(out_e[t - 1, ot], outb[:, ot, :])
                if not last:
                    ACT.dma_start(
                        out_e[t - 1].rearrange("ct p n -> p ct n"), outb[:])

            for t in range(1, T + 1):
                if t == 1:
                    xin = xin1
                else:
                    xin = xtin_pool.tile([128, CT, N], f32r, name="xtin",
                                         tag="xtin", bufs=2)
                    for kt in range(CT):
                        nc.sync.dma_start(xin[:, kt, :], xb[t - 1, kt])
                # proj: per head-pair DR (contraction 2x64 channels), hi and
                # residual weight streams
                x2 = {}
                stps = []
                for ot in range(CT):
                    ps = psM.tile([128, N], f32, name="psp", tag="psp", bufs=2)
                    cin = cp.get(ot)
                    i = 0
                    for hp in range(CT):
                        for lvl in range(2):
                            i += 1
                            nc.tensor.matmul(
                                ps[:],
                                pwt[:, hp, lvl, :, ot * 128:(ot + 1) * 128],
                                os_[t - 1, hp][:], start=(i == 1),
                                stop=(i == 8) and cin is None, perf_mode=DR)
                    if cin is not None:
                        nc.tensor.matmul(ps[:], ids[t], cin[:],
                                         start=False, stop=True)
                    A = work.tile([128, 512], f32, name="lifA", tag="lifA",
                                  bufs=6)
                    ACT.activation(A[:], ps[:], Act.Identity,
                                   bias=bias_sb["pb"][:, t - 1, ot:ot + 1],
                                   scale=float(2.0 ** (t - 1)))
                    stp = mwork.tile([128, N], fp16, name="spk_p",
                                     tag="spk_p", bufs=2)
                    stps.append(stp)
                    spec = (A[:], thr_sb["pb"][:, t - 1, ot:ot + 1],
                            stp, mwork, f"cp_{ot}" if t < T else None,
                            cp, ot)
                    emit_spikes([spec])
                    emit_carries_stt([spec])
                x2b = mwork.tile([128, CT, N], fp16, name="x2t", tag="x2t",
                                 bufs=2)
                for ot in range(CT):
                    POOL.tensor_tensor(x2b[:, ot, :], xin[:, ot, :],
                                       stps[ot][:], Alu.add)
                    x2[ot] = x2b[:, ot, :]
                x2_all[t] = x2
                if t > 1:
                    fc2_wave(t - 1)
                htile = {}
                for ot in range(HT):
                    ps = psM.tile([128, N], f32, name="psf1", tag="psf1",
                                  bufs=4)
                    cin = c1.get(ot)
                    for kt in range(CT):
                        nc.tensor.matmul(
                            ps[:], w1t[:, kt, ot * 128:(ot + 1) * 128],
                            x2[kt], start=(kt == 0), stop=(kt == CT - 1))
                    A = work.tile([128, 512], f32, name="lifA", tag="lifA",
                                  bufs=6)
                    ACT.activation(A[:], ps[:], Act.Identity,
                                   bias=bias_sb["b1"][:, t - 1, ot:ot + 1],
                                   scale=float(2.0 ** (t - 1)))
                    if cin is not None:  # carry-add on Pool, off the PE
                        POOL.tensor_tensor(A[:], cin[:], A[:], Alu.add)
                    st = mwork.tile([128, N], fp8, name="spk_h", tag="spk_h",
                                    bufs=HT)
                    htile[ot] = st
                    spec = (A[:], float(2.0 ** t), st, mwork,
                            f"c1_{ot}" if t < T else None, c1, ot)
                    emit_spikes([spec])
                    emit_carries_stt([spec])
                ht_all[t] = htile
            fc2_wave(T)

    nc.finalize()
    return nc


def _host_prep(inputs):
    def fold(w, b, bn):
        g, bb, m, v = [bn[i].astype(np.float64) for i in range(4)]
        A = g / np.sqrt(v + EPS)
        W = w.astype(np.float64) * A[:, None]
        bias = (b.astype(np.float64) - m) * A + bb
        return W, bias

    def col_layout(WT, dt):
        # [ci, co] -> [128, ci//128, co]
        ci, co = WT.shape
        return np.ascontiguousarray(
            WT.reshape(ci // 128, 128, co).transpose(1, 0, 2).astype(dt))

    def bias_layout(vals):
        # index t-1 holds vals * 2^(t-1): [co] -> [128, T, co//128]
        co = vals.shape[0]
        arr = np.stack([(vals * (2.0 ** t)).reshape(co // 128, 128).T
                        for t in range(T)], axis=1)
        return np.ascontiguousarray(arr.astype(np.float32))

    feed = {}
    wstack = {}
    biases = np.zeros((128, T, 32), np.float32)
    thrs = np.zeros((128, T, 8), np.float32)
    bslot = {"qw": 0, "kw": 4, "pw": 8, "w2": 12, "w1": 16}
    tslot = {"pw": 0, "w2": 4}
    for nm, bkey, bnkey in [("qw", "qb", "qbn"), ("kw", "kb", "kbn"),
                            ("vw", "vb", "vbn"), ("pw", "pb", "pbn"),
                            ("w1", "b1", "bn1"), ("w2", "b2", "bn2")]:
        W, bias = fold(inputs[nm], inputs[bkey], inputs[bnkey])
        if nm == "vw":
            feed["wqkv_v"] = col_layout(W.T, np.float32)
            vrow = np.zeros((1, C + 128), np.float32)
            vrow[0, :C] = bias.astype(np.float32)
            vrow[0, C:] = 1.0
            feed["vrow"] = vrow
            continue
        if nm in ("pw", "w2"):
            # per-channel scale, e4m3 + residual packing
            s = 1.0 / np.sqrt(np.mean(W ** 2, axis=1))
            Ws = (W * s[:, None]).astype(np.float32)
            WT = np.ascontiguousarray(Ws.T)  # [ci, co]
            ci, co = WT.shape
            Q = WT.astype(E4)
            R = (WT - Q.astype(np.float32)).astype(E4)
            if nm == "w2":
                pair = np.stack([Q, R], axis=1)  # [ci, 2, co]
                feed[nm] = np.ascontiguousarray(
                    pair.reshape(ci // 128, 128, 2, co)
                    .transpose(1, 0, 2, 3).astype(E4))
            else:
                # proj: [64, CT(hp), 2(lvl), 2(head), co]
                arr = np.empty((64, CT, 2, 2, co), E4)
                for hp in range(CT):
                    arr[:, hp, 0] = Q[hp * 128:(hp + 1) * 128].reshape(
                        2, 64, co).transpose(1, 0, 2)
                    arr[:, hp, 1] = R[hp * 128:(hp + 1) * 128].reshape(
                        2, 64, co).transpose(1, 0, 2)
                feed[nm] = np.ascontiguousarray(arr)
            # extract bias = s_c*b*2^(t-1); spike thr = s_c*2^t = (2 s_c)*2^(t-1)
            sb = (bias * s).astype(np.float64)
            biases[:, :, bslot[nm]:bslot[nm] + co // 128] = bias_layout(sb)
            thrs[:, :, tslot[nm]:tslot[nm] + co // 128] = \
                bias_layout(2.0 * s.astype(np.float64))
            continue
        wl = col_layout(W.T, np.float16 if nm == "w1" else np.float32)
        if nm in ("qw", "kw"):
            wstack[nm] = wl
        else:
            feed[nm] = wl
        co = bias.shape[0]
        biases[:, :, bslot[nm]:bslot[nm] + co // 128] = bias_layout(bias)
    feed["biases"] = biases
    feed["thrs"] = thrs
    feed["wqkv"] = np.ascontiguousarray(
        np.concatenate([wstack["qw"], wstack["kw"], feed.pop("wqkv_v")],
                       axis=1))

    gamma = 1.0 - 2.0 ** (-5.0 - np.arange(H, dtype=np.float64))
    idx = np.arange(N, dtype=np.float64)
    dist = np.abs(idx[:, None] - idx[None, :])
    scale = (C // H) ** -0.5
    dm = np.empty((H, 128, NT, N), np.float16)
    for h in range(H):
        dm[h] = ((gamma[h] ** dist) * scale * 0.5).reshape(
            NT, 128, N).transpose(1, 0, 2).astype(np.float16)
    feed["dmat"] = dm

    idm = np.zeros((128, 3, 128), np.float32)
    for i, sc in enumerate((0.5, 0.25, 0.125)):
        idm[:, i, :] = sc * np.eye(128, dtype=np.float32)
    feed["idm"] = idm
    return feed


def kernel(**inputs):
    if "nc" not in _CACHE:
        _CACHE["nc"] = _build()
    nc = _CACHE["nc"]
    feed = _host_prep(inputs)
    x = inputs["x"]
    in_maps = []
    for b in range(B):
        m = dict(feed)
        xt = x[:, b].transpose(0, 2, 1).reshape(T, CT, 128, N)
        m["xb"] = np.ascontiguousarray(xt)
        in_maps.append(m)
    res = None
    last_err = None
    for _attempt in range(3):
        try:
            res = run_bass_kernel_spmd(nc, in_maps, list(range(B)))
            break
        except Exception as e:  # transient NRT device wedges recover on retry
            last_err = e
    if res is None:
        raise last_err
    out = np.empty((T, B, N, C), np.float32)
    for b in range(B):
        oT = res.results[b]["out"].reshape(T, C, N).astype(np.float32)
        out[:, b] = oT.transpose(0, 2, 1)
    return out


# revision 27
# speedup vs baseline: 1.0163x; 1.0163x over previous
"""Spiking transformer block (SpikingRetention + spiking MLP) on 8 Trainium2
cores. Data-parallel over B=8 (one batch element per NeuronCore).

Key design (v2):
- Binary spikes are exact in fp8e4, enabling DoubleRow (double-pumped) PE
  matmuls at 0.5 cycles/row:
  * scores q.T k: stride-0 dim-2 APs compute 2*(k.T q) exactly; the decay
    matrix folds the 0.5.
  * retention out: real 2-chunk DoubleRow over m-tiles into per-head-pair
    [64, 2, N] psum (DR dst must start at partition 0 on HW).
  * proj: retention spikes are [64(d), 2(head), N] tiles; each DR contracts a
    128-channel head pair; hi and residual weight streams are separate DRs.
  * fc2: weights packed as [Q(W s), e4m3-residual] chunk pairs with a
    stride-0 spike ifmap. Per-channel scales s_c keep quantization ~2^-4.
- LIF carry-adds ride the PE as scaled-identity accumulation matmuls into the
  next wave's psum group.
- LIF per step: Act extract (A = 2^(t-1) psum + b~); spike on DVE (is_ge) or
  Act (Relu+Sign); stage-A carry = A * (A < thr) computed as complement mask
  (DVE is_lt -> fp16) times A on Pool (gpsimd cannot touch PSUM or run
  tensor_scalar); stage-B carry = DVE scalar_tensor_tensor.

Membrane algebra: A_t = 2^t u_t = A^r_{t-1} + 2^(t-1)(Wx_t + b). Carry
C = A (A < th 2^t); the consuming wave-t psum gets 2^-(t-1) I @ C. proj/fc2
run entirely in per-channel-scaled units (psum, bias, threshold, carry all
scaled by s_c), so no rescale is ever needed.
"""

from contextlib import ExitStack

import numpy as np
import ml_dtypes

import concourse.bacc as bacc
import concourse.tile as tile
from concourse import mybir
from concourse.bass_utils import run_bass_kernel_spmd

f32 = mybir.dt.float32
f32r = mybir.dt.float32r
fp16 = mybir.dt.float16
fp8 = mybir.dt.float8e4
Alu = mybir.AluOpType
Act = mybir.ActivationFunctionType
DR = mybir.MatmulPerfMode.DoubleRow

E4 = ml_dtypes.float8_e4m3

T, B, N, C = 4, 8, 512, 512
HID = 2048
H = 8
EPS = 1e-5
NT = N // 128
CT = C // 128
HT = HID // 128

_CACHE = {}


def _dr2(ap):
    """[p, f] -> [p, 2(stride0), f] for stride-0 DoubleRow operands."""
    p, fr = ap.shape
    return ap.unsqueeze(1).broadcast_to([p, 2, fr])


def _build():
    nc = bacc.Bacc("TRN2", target_bir_lowering=False, debug=False)

    xb = nc.declare_dram_parameter("xb", [T, CT, 128, N], f32r, isOutput=False)
    wqkv_e = nc.declare_dram_parameter("wqkv", [128, 3 * CT, C], f32r,
                                       isOutput=False)
    pw_e = nc.declare_dram_parameter("pw", [64, CT, 2, 2, C], fp8,
                                     isOutput=False)
    w1_e = nc.declare_dram_parameter("w1", [128, CT, HID], fp16, isOutput=False)
    w2_e = nc.declare_dram_parameter("w2", [128, HT, 2, C], fp8, isOutput=False)
    bias_e = nc.declare_dram_parameter("biases", [128, T, 32], f32,
                                       isOutput=False)
    thr_e = nc.declare_dram_parameter("thrs", [128, T, 8], f32, isOutput=False)
    vb_e = nc.declare_dram_parameter("vrow", [1, C + 128], f32r, isOutput=False)
    dmat_e = nc.declare_dram_parameter("dmat", [H, 128, NT, N], fp16,
                                       isOutput=False)
    idm_e = nc.declare_dram_parameter("idm", [128, 3, 128], f32r,
                                      isOutput=False)
    out_e = nc.declare_dram_parameter("out", [T, CT, 128, N], fp16,
                                      isOutput=True)

    DVE = nc.vector
    POOL = nc.gpsimd
    ACT = nc.scalar

    with tile.TileContext(nc) as tc, ExitStack() as ctx:
        pers = ctx.enter_context(tc.tile_pool(name="pers", bufs=1))
        work = ctx.enter_context(tc.tile_pool(name="work", bufs=1))
        xa_pool = ctx.enter_context(tc.tile_pool(name="xa_pool", bufs=1))
        spk_o_pool = ctx.enter_context(tc.tile_pool(name="spk_o_pool", bufs=1))
        wmlp_pool = ctx.enter_context(tc.tile_pool(name="wmlp_pool", bufs=1))
        pwt = wmlp_pool.tile([64, CT, 2, 2, C], fp8, name="w_pw")

        ball = pers.tile([128, T, 32], f32, name="ball")
        thrt = pers.tile([128, T, 8], f32, name="thrt")
        vrow = pers.tile([1, C + 128], f32r, name="vrow")
        idmt = pers.tile([128, 3, 128], f32r, name="idmt")
        ACT.dma_start(ball[:], bias_e[:, :, :])
        ACT.dma_start(vrow[:], vb_e[:, :])
        bias_sb = {"qb": ball[:, :, 0:4], "kb": ball[:, :, 4:8],
                   "pb": ball[:, :, 8:12], "b2": ball[:, :, 12:16],
                   "b1": ball[:, :, 16:32]}
        thr_sb = {"pb": thrt[:, :, 0:4], "b2": thrt[:, :, 4:8]}
        nthr = {}
        for tv in (1.0, 2.0, 4.0, 8.0, 16.0):
            tt_ = pers.tile([128, 1], f32, name=f"nthr{int(tv)}")
            nc.vector.memset(tt_[:], -tv)
            nthr[tv] = tt_
        vbrow = vrow[:, 0:C]
        ones128 = vrow[:, C:C + 128]
        ids = {tt: idmt[:, tt - 2, :] for tt in (2, 3, 4)}

        os_ = {}
        decay_rr = [0]

        # ---------------- LIF helpers ----------------
        def act_spike(st_ap, src, tv, shape=None, p0=128, pool=None):
            # spike via Act (Relu with negated threshold, then Sign)
            shape = shape or [128, 512]
            rl = pool.tile(shape, f32, name="lifrl",
                           tag="lifrl" if p0 == 128 else "lifrlr", bufs=2)
            ACT.activation(rl[:], src, Act.Relu, bias=nthr[tv][0:p0, 0:1])
            ACT.activation(st_ap, rl[:], Act.Sign)

        def emit_spikes(specs):
            for (src, thr, st, cp, ctag, cdst, ckey) in specs:
                if st is not None:
                    DVE.tensor_scalar(st[:], src, thr, None, Alu.is_ge)

        def carry_sbar(src, thr, cp, ctag, cdst, ckey, shape=None):
            # stage-A carry: sbar = (A < thr) fp16 on DVE, C = A*sbar on Pool
            shape = shape or [128, 512]
            p0 = shape[0]
            sb = cp.tile(shape, fp16, name="sbar",
                         tag="sbar" if p0 == 128 else "sbarr", bufs=3)
            DVE.tensor_scalar(sb[:], src, thr, None, Alu.is_lt)
            cn = cp.tile(shape, f32r, name="lifC", tag=ctag, bufs=1)
            POOL.tensor_tensor(cn[:], src, sb[:], Alu.mult)
            cdst[ckey] = cn

        def emit_carries_stt(specs):
            # stage-B carry on DVE: C = (A < thr) * A in one op
            for (src, thr, st, cp, ctag, cdst, ckey) in specs:
                if ctag is not None:
                    cn = cp.tile([128, 512], f32r, name="lifC", tag=ctag,
                                 bufs=1)
                    DVE.scalar_tensor_tensor(cn[:], src, thr, src,
                                             Alu.is_lt, Alu.mult)
                    cdst[ckey] = cn

        # =========== stage A: qkv + retention, t-outer wavefront ===========
        with tc.tile_pool(name="wqkv_pool", bufs=1) as wqkv_pool, \
             tc.tile_pool(name="spk_pool", bufs=1) as spk_pool, \
             tc.tile_pool(name="carry_pool", bufs=1) as carry_pool, \
             tc.tile_pool(name="dm_pool", bufs=1) as dm_pool, \
             tc.tile_pool(name="spool", bufs=1) as spool, \
             tc.tile_pool(name="psA", bufs=1, space="PSUM") as psA:
            wqkv_t = wqkv_pool.tile([128, 3 * CT, C], f32r, name="w_qkv")
            # startup: interleave x wave-1 chunks with qw chunks so the first
            # matmul can start after ~0.5MB of DMA; all on the Pool SWDGE
            # queue (served in emission order by the DMA device).
            xwt = xa_pool.tile([128, CT, N], f32r, name="xT", tag="xT", bufs=2)
            for kt in range(CT):
                nc.sync.dma_start(xwt[:, kt, :], xb[0, kt])
                nc.sync.dma_start(wqkv_t[:, kt, :], wqkv_e[:, kt, :])
            for kt in range(CT, 3 * CT):
                nc.sync.dma_start(wqkv_t[:, kt, :], wqkv_e[:, kt, :])
            wq = {nm: wqkv_t[:, i * CT:(i + 1) * CT, :]
                  for i, nm in enumerate(("qw", "kw", "vw"))}
            dmt = dm_pool.tile([128, H, NT, N], fp16, name="dmt")
            dms = [dmt[:, h] for h in range(H)]

            cq = {}     # carries for q/k/v chains, keyed (nm, ot)
            c_ret = {}  # retention carries per hp

            def ret_scores(hp, qs_p, ks_p, sdst):
                # per head pair: 8 stride-0 DR matmuls + 4 decay multiplies
                h0, h1 = 2 * hp, 2 * hp + 1
                for half in range(2):
                    ps0 = psA.tile([128, 2, N], f32, name="sc0", tag="sc0",
                                   bufs=1)
                    ps1 = psA.tile([128, 2, N], f32, name="sc1", tag="sc1",
                                   bufs=1)
                    for j in range(2):
                        mt = 2 * half + j
                        nc.tensor.matmul(
                            ps0[:, j, :],
                            _dr2(ks_p[hp][0:64, mt * 128:(mt + 1) * 128]),
                            _dr2(qs_p[hp][0:64, :]),
                            start=True, stop=True, perf_mode=DR)
                        nc.tensor.matmul(
                            ps1[:, j, :],
                            _dr2(ks_p[hp][64:128, mt * 128:(mt + 1) * 128]),
                            _dr2(qs_p[hp][64:128, :]),
                            start=True, stop=True, perf_mode=DR)
                    def decay(ps, h):
                        s_ = spool.tile([128, 2, N], fp8, name="sd",
                                        tag=f"sd{decay_rr[0] % 3}", bufs=2)
                        if False:
                            # offload via Act copy (scores are small ints,
                            # exact in fp16) + Pool multiply
                            cpy = spool.tile([128, 2, N], fp16, name="scp",
                                             tag="scp", bufs=2)
                            ACT.activation(cpy[:], ps[:], Act.Copy, bias=0.0,
                                           scale=1.0)
                            POOL.tensor_tensor(
                                s_[:], cpy[:],
                                dms[h][:, 2 * half:2 * half + 2, :], Alu.mult)
                        else:
                            DVE.tensor_tensor(
                                s_[:], ps[:],
                                dms[h][:, 2 * half:2 * half + 2, :], Alu.mult)
                        decay_rr[0] += 1
                        return s_
                    s0 = decay(ps0, h0)
                    s1 = decay(ps1, h1)
                    sdst[hp, half] = (s0, s1)

            def ret_out(hp, sdst, vt_p, t_r):
                h0, h1 = 2 * hp, 2 * hp + 1
                pso = psA.tile([64, 2, N], f32, name="pso", tag="pso", bufs=1)
                has_c = (hp in c_ret)
                for half in range(2):
                    s0, s1 = sdst.pop((hp, half))
                    last = (half == 1) and not has_c
                    nc.tensor.matmul(
                        pso[:, 0, :],
                        vt_p[:, 2 * half:2 * half + 2,
                             h0 * 64:(h0 + 1) * 64],
                        s0[:], start=(half == 0), stop=last, perf_mode=DR)
                    nc.tensor.matmul(
                        pso[:, 1, :],
                        vt_p[:, 2 * half:2 * half + 2,
                             h1 * 64:(h1 + 1) * 64],
                        s1[:], start=(half == 0), stop=last, perf_mode=DR)
                if has_c:
                    cr = c_ret[hp]
                    for j in range(2):
                        nc.tensor.matmul(pso[:, j, :], ids[t_r][0:64, 0:64],
                                         cr[:, j, :], start=False, stop=True)
                st = spk_o_pool.tile([64, 2, N], fp8, name="spk_os",
                                     tag="spk_os", bufs=16)
                os_[t_r - 1, hp] = st
                A = spool.tile([64, 2, 512], f32, name="lifAr", tag="lifAr",
                               bufs=2)
                ACT.activation(A[:], pso[:], Act.Copy, bias=0.0,
                               scale=float(2.0 ** (t_r - 1)))
                act_spike(st[:], A[:], float(2.0 ** (t_r - 1)),
                          shape=[64, 2, 512], p0=64, pool=spool)
                if t_r < T:
                    carry_sbar(A[:], float(2.0 ** (t_r - 1)), spool,
                               f"c_o{hp}", c_ret, hp, shape=[64, 2, 512])

            prev = None
            xw_next = None
            for t in range(1, T + 1):
                if t > 1:
                    xwt = xw_next
                xw = {ct: xwt[:, ct, :] for ct in range(CT)}
                qs_c = {}
                ks_c = {}
                vt = spk_pool.tile([128, NT, C], fp8, name="vn", tag="vn",
                                   bufs=2)

                def emit_qk(nm, bnm, dst, ot, t=t):
                    ps = psA.tile([128, N], f32, name="psq", tag="psq", bufs=2)
                    cin = cq.get((nm, ot))
                    for kt in range(CT):
                        nc.tensor.matmul(
                            ps[:], wq[nm][:, kt, ot * 128:(ot + 1) * 128],
                            xw[kt], start=(kt == 0),
                            stop=(kt == CT - 1) and cin is None)
                    if cin is not None:
                        nc.tensor.matmul(ps[:], ids[t], cin[:],
                                         start=False, stop=True)
                    A = work.tile([128, 512], f32, name="lifA", tag="lifA",
                                  bufs=6)
                    ACT.activation(A[:], ps[:], Act.Identity,
                                   bias=bias_sb[bnm][:, t - 1, ot:ot + 1],
                                   scale=float(2.0 ** (t - 1)))
                    st = spk_pool.tile([128, N], fp8, name=f"spk_{nm}",
                                       tag=f"spk_{nm}", bufs=8)
                    dst[ot] = st
                    DVE.tensor_scalar(st[:], A[:], float(2.0 ** t), None,
                                      Alu.is_ge)
                    if t < T:
                        carry_sbar(A[:], float(2.0 ** t), carry_pool,
                                   f"c_{nm}{ot}", cq, (nm, ot))

                def emit_v(nt, t=t):
                    ps = psA.tile([128, C], f32, name="psv", tag="psq", bufs=2)
                    cin = cq.get(("vw", nt))
                    for kt in range(CT):
                        nc.tensor.matmul(ps[:],
                                         xw[kt][:, nt * 128:(nt + 1) * 128],
                                         wq["vw"][:, kt, :],
                                         start=(kt == 0), stop=False)
                    nc.tensor.matmul(ps[:], ones128, vbrow,
                                     start=False, stop=cin is None)
                    if cin is not None:
                        nc.tensor.matmul(ps[:], ids[t], cin[:],
                                         start=False, stop=True)
                    A = work.tile([128, 512], f32, name="lifA", tag="lifA",
                                  bufs=6)
                    ACT.activation(A[:], ps[:], Act.Copy, bias=0.0,
                                   scale=float(2.0 ** (t - 1)))
                    DVE.tensor_scalar(vt[:, nt, :], A[:], float(2.0 ** t),
                                      None, Alu.is_ge)
                    if t < T:
                        carry_sbar(A[:], float(2.0 ** t), carry_pool,
                                   f"c_vw{nt}", cq, ("vw", nt))

                groups = [lambda ot=ot: emit_qk("qw", "qb", qs_c, ot)
                          for ot in range(CT)]
                groups += [lambda ot=ot: emit_qk("kw", "kb", ks_c, ot)
                           for ot in range(CT)]
                groups += [lambda nt=nt: emit_v(nt) for nt in range(NT)]

                if prev is not None:
                    qs_p, ks_p, vt_p = prev
                    sd = {}
                    order = [0, 1, 2, ("s", 0), 3, 4, ("o", 0), ("s", 1),
                             5, 6, ("o", 1), ("s", 2), 7, 8, ("o", 2),
                             ("s", 3), 9, 10, ("o", 3), 11]
                    for item in order:
                        if isinstance(item, int):
                            groups[item]()
                        elif item[0] == "s":
                            ret_scores(item[1], qs_p, ks_p, sd)
                        else:
                            ret_out(item[1], sd, vt_p, t - 1)
                else:
                    for g in groups:
                        g()
                if t < T:  # prefetch next wave's x (SP hwdge queue: free)
                    xw_next = xa_pool.tile([128, CT, N], f32r, name="xT",
                                           tag="xT", bufs=2)
                    for kt in range(CT):
                        nc.sync.dma_start(xw_next[:, kt, :], xb[t, kt])
                if t == 1:  # decay matrices after wave-2 x
                    for hp in range(4):
                        nc.sync.dma_start(
                            dmt[:, 2 * hp:2 * hp + 2],
                            dmat_e.rearrange("h p nt n -> p h nt n")
                            [:, 2 * hp:2 * hp + 2])
                if t == 1:  # proj weights are small; land them early
                    ACT.dma_start(idmt[:], idm_e[:, :, :])
                    ACT.dma_start(thrt[:], thr_e[:, :, :])
                    nc.sync.dma_start(pwt[:], pw_e[:, :, :, :, :])
                prev = (qs_c, ks_c, vt)
            # final retention wave (t = T)
            qs_p, ks_p, vt_p = prev
            sd = {}
            for hp in range(H // 2):
                ret_scores(hp, qs_p, ks_p, sd)
                ret_out(hp, sd, vt_p, T)

        # =========== stage B: proj + MLP + output ===========
        with tc.tile_pool(name="wmlp2", bufs=1) as wmlp2, \
             tc.tile_pool(name="mwork", bufs=1) as mwork, \
             tc.tile_pool(name="xtin_pool", bufs=1) as xtin_pool, \
             tc.tile_pool(name="psM", bufs=1, space="PSUM") as psM:
            w1t = wmlp2.tile([128, CT, HID], fp16, name="w_w1")
            w2t = wmlp2.tile([128, HT, 2, C], fp8, name="w_w2")
            xin1 = xtin_pool.tile([128, CT, N], f32r, name="xtin",
                                  tag="xtin", bufs=2)
            for kt in range(CT):
                nc.sync.dma_start(xin1[:, kt, :], xb[0, kt])
            for kt in range(CT):
                nc.sync.dma_start(w1t[:, kt], w1_e[:, kt])
            for ktp in range(4):
                nc.sync.dma_start(w2t[:, 4 * ktp:4 * ktp + 4],
                                  w2_e[:, 4 * ktp:4 * ktp + 4])
            cp = {}
            c1 = {}
            c2 = {}
            x2_all = {}
            ht_all = {}

            def fc2_wave(t):
                htile = ht_all.pop(t)
                x2 = x2_all.pop(t)
                last = (t == T)
                sts = []
                for ot in range(CT):
                    ps = psM.tile([128, N], f32, name="psf2", tag="psf2",
                                  bufs=2)
                    cin = c2.get(ot)
                    for kt in range(HT):
                        nc.tensor.matmul(
                            ps[:], w2t[:, kt, :, ot * 128:(ot + 1) * 128],
                            _dr2(htile[kt][:]), start=(kt == 0),
                            stop=(kt == HT - 1) and cin is None, perf_mode=DR)
                    if cin is not None:
                        nc.tensor.matmul(ps[:], ids[t], cin[:],
                                         start=False, stop=True)
                    A = work.tile([128, 512], f32, name="lifA", tag="lifA",
                                  bufs=6)
                    ACT.activation(A[:], ps[:], Act.Identity,
                                   bias=bias_sb["b2"][:, t - 1, ot:ot + 1],
                                   scale=float(2.0 ** (t - 1)))
                    st = mwork.tile([128, N], fp16, name="spk_m", tag="spk_m",
                                    bufs=2)
                    sts.append(st)
                    spec = (A[:], thr_sb["b2"][:, t - 1, ot:ot + 1], st,
                            mwork, f"c2_{ot}" if t < T else None, c2, ot)
                    emit_spikes([spec])
                    emit_carries_stt([spec])
                outb = mwork.tile([128, CT, N], fp16, name="outb", tag="outb",
                                  bufs=1)
                for ot in range(CT):
                    DVE.tensor_tensor(outb[:, ot, :], x2[ot], sts[ot][:],
                                      Alu.add)
                    if last:
                        nc.sync.dma_start(out_e[t - 1, ot], outb[:, ot, :])
                if not last:
                    ACT.dma_start(
                        out_e[t - 1].rearrange("ct p n -> p ct n"), outb[:])

            for t in range(1, T + 1):
                if t == 1:
                    xin = xin1
                else:
                    xin = xtin_pool.tile([128, CT, N], f32r, name="xtin",
                                         tag="xtin", bufs=2)
                    for kt in range(CT):
                        nc.sync.dma_start(xin[:, kt, :], xb[t - 1, kt])
                # proj: per head-pair DR (contraction 2x64 channels), hi and
                # residual weight streams
                x2 = {}
                stps = []
                for ot in range(CT):
                    ps = psM.tile([128, N], f32, name="psp", tag="psp", bufs=2)
                    cin = cp.get(ot)
                    i = 0
                    for hp in range(CT):
                        for lvl in range(2):
                            i += 1
                            nc.tensor.matmul(
                                ps[:],
                                pwt[:, hp, lvl, :, ot * 128:(ot + 1) * 128],
                                os_[t - 1, hp][:], start=(i == 1),
                                stop=(i == 8) and cin is None, perf_mode=DR)
                    if cin is not None:
                        nc.tensor.matmul(ps[:], ids[t], cin[:],
                                         start=False, stop=True)
                    A = work.tile([128, 512], f32, name="lifA", tag="lifA",
                                  bufs=6)
                    ACT.activation(A[:], ps[:], Act.Identity,
                                   bias=bias_sb["pb"][:, t - 1, ot:ot + 1],
                                   scale=float(2.0 ** (t - 1)))
                    stp = mwork.tile([128, N], fp16, name="spk_p",
                                     tag="spk_p", bufs=2)
                    stps.append(stp)
                    spec = (A[:], thr_sb["pb"][:, t - 1, ot:ot + 1],
                            stp, mwork, f"cp_{ot}" if t < T else None,
                            cp, ot)
                    emit_spikes([spec])
                    emit_carries_stt([spec])
                x2b = mwork.tile([128, CT, N], fp16, name="x2t", tag="x2t",
                                 bufs=2)
                for ot in range(CT):
                    POOL.tensor_tensor(x2b[:, ot, :], xin[:, ot, :],
                                       stps[ot][:], Alu.add)
                    x2[ot] = x2b[:, ot, :]
                x2_all[t] = x2
                if t > 1:
                    fc2_wave(t - 1)
                htile = {}
                for ot in range(HT):
                    ps = psM.tile([128, N], f32, name="psf1", tag="psf1",
                                  bufs=4)
                    cin = c1.get(ot)
                    for kt in range(CT):
                        nc.tensor.matmul(
                            ps[:], w1t[:, kt, ot * 128:(ot + 1) * 128],
                            x2[kt], start=(kt == 0), stop=(kt == CT - 1))
                    A = work.tile([128, 512], f32, name="lifA", tag="lifA",
                                  bufs=6)
                    ACT.activation(A[:], ps[:], Act.Identity,
                                   bias=bias_sb["b1"][:, t - 1, ot:ot + 1],
                                   scale=float(2.0 ** (t - 1)))
                    if cin is not None:  # carry-add on Pool, off the PE
                        POOL.tensor_tensor(A[:], cin[:], A[:], Alu.add)
                    st = mwork.tile([128, N], fp8, name="spk_h", tag="spk_h",
                                    bufs=HT)
                    htile[ot] = st
                    spec = (A[:], float(2.0 ** t), st, mwork,
                            f"c1_{ot}" if t < T else None, c1, ot)
                    emit_spikes([spec])
                    emit_carries_stt([spec])
                ht_all[t] = htile
            fc2_wave(T)

    nc.finalize()
    return nc


def _host_prep(inputs):
    def fold(w, b, bn):
        g, bb, m, v = [bn[i].astype(np.float64) for i in range(4)]
        A = g / np.sqrt(v + EPS)
        W = w.astype(np.float64) * A[:, None]
        bias = (b.astype(np.float64) - m) * A + bb
        return W, bias

    def col_layout(WT, dt):
        # [ci, co] -> [128, ci//128, co]
        ci, co = WT.shape
        return np.ascontiguousarray(
            WT.reshape(ci // 128, 128, co).transpose(1, 0, 2).astype(dt))

    def bias_layout(vals):
        # index t-1 holds vals * 2^(t-1): [co] -> [128, T, co//128]
        co = vals.shape[0]
        arr = np.stack([(vals * (2.0 ** t)).reshape(co // 128, 128).T
                        for t in range(T)], axis=1)
        return np.ascontiguousarray(arr.astype(np.float32))

    feed = {}
    wstack = {}
    biases = np.zeros((128, T, 32), np.float32)
    thrs = np.zeros((128, T, 8), np.float32)
    bslot = {"qw": 0, "kw": 4, "pw": 8, "w2": 12, "w1": 16}
    tslot = {"pw": 0, "w2": 4}
    for nm, bkey, bnkey in [("qw", "qb", "qbn"), ("kw", "kb", "kbn"),
                            ("vw", "vb", "vbn"), ("pw", "pb", "pbn"),
                            ("w1", "b1", "bn1"), ("w2", "b2", "bn2")]:
        W, bias = fold(inputs[nm], inputs[bkey], inputs[bnkey])
        if nm == "vw":
            feed["wqkv_v"] = col_layout(W.T, np.float32)
            vrow = np.zeros((1, C + 128), np.float32)
            vrow[0, :C] = bias.astype(np.float32)
            vrow[0, C:] = 1.0
            feed["vrow"] = vrow
            continue
        if nm in ("pw", "w2"):
            # per-channel scale, e4m3 + residual packing
            s = 1.0 / np.sqrt(np.mean(W ** 2, axis=1))
            Ws = (W * s[:, None]).astype(np.float32)
            WT = np.ascontiguousarray(Ws.T)  # [ci, co]
            ci, co = WT.shape
            Q = WT.astype(E4)
            R = (WT - Q.astype(np.float32)).astype(E4)
            if nm == "w2":
                pair = np.stack([Q, R], axis=1)  # [ci, 2, co]
                feed[nm] = np.ascontiguousarray(
                    pair.reshape(ci // 128, 128, 2, co)
                    .transpose(1, 0, 2, 3).astype(E4))
            else:
                # proj: [64, CT(hp), 2(lvl), 2(head), co]
                arr = np.empty((64, CT, 2, 2, co), E4)
                for hp in range(CT):
                    arr[:, hp, 0] = Q[hp * 128:(hp + 1) * 128].reshape(
                        2, 64, co).transpose(1, 0, 2)
                    arr[:, hp, 1] = R[hp * 128:(hp + 1) * 128].reshape(
                        2, 64, co).transpose(1, 0, 2)
                feed[nm] = np.ascontiguousarray(arr)
            # extract bias = s_c*b*2^(t-1); spike thr = s_c*2^t = (2 s_c)*2^(t-1)
            sb = (bias * s).astype(np.float64)
            biases[:, :, bslot[nm]:bslot[nm] + co // 128] = bias_layout(sb)
            thrs[:, :, tslot[nm]:tslot[nm] + co // 128] = \
                bias_layout(2.0 * s.astype(np.float64))
            continue
        wl = col_layout(W.T, np.float16 if nm == "w1" else np.float32)
        if nm in ("qw", "kw"):
            wstack[nm] = wl
        else:
            feed[nm] = wl
        co = bias.shape[0]
        biases[:, :, bslot[nm]:bslot[nm] + co // 128] = bias_layout(bias)
    feed["biases"] = biases
    feed["thrs"] = thrs
    feed["wqkv"] = np.ascontiguousarray(
        np.concatenate([wstack["qw"], wstack["kw"], feed.pop("wqkv_v")],
                       axis=1))

    gamma = 1.0 - 2.0 ** (-5.0 - np.arange(H, dtype=np.float64))
    idx = np.arange(N, dtype=np.float64)
    dist = np.abs(idx[:, None] - idx[None, :])
    scale = (C // H) ** -0.5
    dm = np.empty((H, 128, NT, N), np.float16)
    for h in range(H):
        dm[h] = ((gamma[h] ** dist) * scale * 0.5).reshape(
            NT, 128, N).transpose(1, 0, 2).astype(np.float16)
    feed["dmat"] = dm

    idm = np.zeros((128, 3, 128), np.float32)
    for i, sc in enumerate((0.5, 0.25, 0.125)):
        idm[:, i, :] = sc * np.eye(128, dtype=np.float32)
    feed["idm"] = idm
    return feed


def kernel(**inputs):
    if "nc" not in _CACHE:
        _CACHE["nc"] = _build()
    nc = _CACHE["nc"]
    feed = _host_prep(inputs)
    x = inputs["x"]
    in_maps = []
    for b in range(B):
        m = dict(feed)
        xt = x[:, b].transpose(0, 2, 1).reshape(T, CT, 128, N)
        m["xb"] = np.ascontiguousarray(xt)
        in_maps.append(m)
    res = None
    last_err = None
    for _attempt in range(3):
        try:
            res = run_bass_kernel_spmd(nc, in_maps, list(range(B)))
            break
        except Exception as e:  # transient NRT device wedges recover on retry
            last_err = e
    if res is None:
        raise last_err
    out = np.empty((T, B, N, C), np.float32)
    for b in range(B):
        oT = res.results[b]["out"].reshape(T, C, N).astype(np.float32)
        out[:, b] = oT.transpose(0, 2, 1)
    return out
